# revision 2
# baseline (speedup 1.0000x reference)
"""Trainium2 Bass kernel for nn_Attention_47605417509124 (sparse_attention).

Reference computation (B=4, N=4096, C=256), per batch b:
    g_x     = x @ g_w.T + g_b
    theta_x = x @ theta_w.T + theta_b
    phi_x   = x @ phi_w.T + phi_b
    f       = phi_x @ theta_x.T / N          # no softmax
    y       = f @ g_x
    out     = y @ W_w.T + W_b + x

Sharding: 8 cores = 4 batches x 2 sequence halves. Each core computes the
full theta/g projections for its batch (redundantly with its pair core) and
the phi rows / score rows / output rows for its own half of the sequence.

All matmuls run in float32r (TF32-like reduced-precision fp32, full PE rate)
with fp32 PSUM accumulation. Host-side (free) prep: transposes of x and the
weights, 1/N folded into g, W_b folded into the residual.
"""

import numpy as np

import concourse.bass as bass
import concourse.mybir as mybir
import concourse.tile as tile
from concourse import bacc
from concourse.bass_utils import run_bass_kernel_spmd

B, N, C = 4, 4096, 256
NCORES = 8
HALF = N // 2  # sequence rows handled per core
P = 128

F32 = mybir.dt.float32
F32R = mybir.dt.float32r
AF = mybir.ActivationFunctionType

_CACHE = {}


def _build_module():
    nc = bacc.Bacc("TRN2", target_bir_lowering=False, debug=False,
                   num_devices=NCORES)

    # ---- external I/O (per-core shapes) ----
    xT_d = nc.dram_tensor("xT", [C, N], F32R, kind="ExternalInput")
    xTR_d = nc.dram_tensor("xTR", [C, HALF], F32R, kind="ExternalInput")
    thW_d = nc.dram_tensor("thW", [C, C], F32R, kind="ExternalInput")
    phW_d = nc.dram_tensor("phW", [C, C], F32R, kind="ExternalInput")
    gW_d = nc.dram_tensor("gW", [C, C], F32R, kind="ExternalInput")
    WW_d = nc.dram_tensor("WW", [C, C], F32R, kind="ExternalInput")
    thb_d = nc.dram_tensor("thb", [P, 2], F32, kind="ExternalInput")
    phb_d = nc.dram_tensor("phb", [P, 2], F32, kind="ExternalInput")
    gbb_d = nc.dram_tensor("gbb", [P, C], F32, kind="ExternalInput")
    resid_d = nc.dram_tensor("resid", [HALF, C], F32, kind="ExternalInput")
    out_d = nc.dram_tensor("out", [HALF, C], F32, kind="ExternalOutput")

    with tile.TileContext(nc) as tc:
        with tc.tile_pool(name="big", bufs=1) as big, \
             tc.tile_pool(name="fT", bufs=3) as fTp, \
             tc.tile_pool(name="ps_work", bufs=4, space="PSUM") as psw, \
             tc.tile_pool(name="ps_acc", bufs=4, space="PSUM") as psa:

            # ---- SBUF residents ----
            xT_sb = big.tile([P, 2, N], F32R)       # x[b].T       32KB/part
            xTR_sb = big.tile([P, 2, HALF], F32R)   # x[b,R].T     16KB/part
            thW_sb = big.tile([P, 2, C], F32R)
            phW_sb = big.tile([P, 2, C], F32R)
            gW_sb = big.tile([P, 2, C], F32R)
            WW_sb = big.tile([P, 2, C], F32R)
            thb_sb = big.tile([P, 2], F32)
            phb_sb = big.tile([P, 2], F32)
            gbb_sb = big.tile([P, C], F32)
            thetaT_sb = big.tile([P, 2, N], F32R)   # theta_x.T    32KB/part
            phiT_sb = big.tile([P, 2, HALF], F32R)  # phi_x.T      16KB/part
            gx_sb = big.tile([P, N // P, C], F32R)  # g_x natural  32KB/part
            yT_sb = big.tile([P, 2, HALF], F32R)    # y.T          16KB/part
            resid_sb = big.tile([P, HALF // P, C], F32)  # also output staging

            # ---- input DMAs (order = consumption order) ----
            nc.sync.dma_start(out=thb_sb, in_=thb_d.ap())
            nc.sync.dma_start(out=phb_sb, in_=phb_d.ap())
            nc.sync.dma_start(
                out=thW_sb, in_=thW_d.ap().rearrange("(o p) d -> p o d", p=P))
            nc.sync.dma_start(
                out=phW_sb, in_=phW_d.ap().rearrange("(o p) d -> p o d", p=P))
            # x.T in 1MB column blocks so stage A can start early
            xT_r = xT_d.ap().rearrange("(o p) n -> p o n", p=P)
            for kb in range(4):
                s = slice(kb * (N // 4), (kb + 1) * (N // 4))
                nc.sync.dma_start(out=xT_sb[:, :, s], in_=xT_r[:, :, s])
            xTR_r = xTR_d.ap().rearrange("(o p) n -> p o n", p=P)
            for kb in range(2):
                s = slice(kb * (HALF // 2), (kb + 1) * (HALF // 2))
                nc.sync.dma_start(out=xTR_sb[:, :, s], in_=xTR_r[:, :, s])
            nc.sync.dma_start(
                out=gW_sb, in_=gW_d.ap().rearrange("(o p) d -> p o d", p=P))
            nc.sync.dma_start(out=gbb_sb, in_=gbb_d.ap())
            nc.sync.dma_start(
                out=WW_sb, in_=WW_d.ap().rearrange("(o p) d -> p o d", p=P))
            resid_r = resid_d.ap().rearrange("(t p) d -> p t d", p=P)
            for kb in range(2):
                s = slice(kb * 8, (kb + 1) * 8)
                nc.sync.dma_start(out=resid_sb[:, s, :], in_=resid_r[:, s, :])

            # ---- stage A: theta_x.T[d, j] = thW.T @ xT (+bias) ----
            for dh in range(2):
                for jc in range(8):
                    ps = psw.tile([P, 512], F32, tag="work")
                    js = slice(jc * 512, (jc + 1) * 512)
                    for ch in range(2):
                        nc.tensor.matmul(
                            ps,
                            thW_sb[:, ch, dh * P:(dh + 1) * P],
                            xT_sb[:, ch, js],
                            start=(ch == 0), stop=(ch == 1))
                    nc.scalar.activation(
                        out=thetaT_sb[:, dh, js], in_=ps, func=AF.Identity,
                        bias=thb_sb[:, dh:dh + 1], scale=1.0)

            # ---- stage A: phi_x.T[d, i] over own rows ----
            for dh in range(2):
                for ic in range(4):
                    ps = psw.tile([P, 512], F32, tag="work")
                    isl = slice(ic * 512, (ic + 1) * 512)
                    for ch in range(2):
                        nc.tensor.matmul(
                            ps,
                            phW_sb[:, ch, dh * P:(dh + 1) * P],
                            xTR_sb[:, ch, isl],
                            start=(ch == 0), stop=(ch == 1))
                    nc.scalar.activation(
                        out=phiT_sb[:, dh, isl], in_=ps, func=AF.Identity,
                        bias=phb_sb[:, dh:dh + 1], scale=1.0)

            # ---- stage A: g_x[j, d] natural layout (g pre-scaled by 1/N) ----
            for jt in range(N // P):
                ps = psw.tile([P, C], F32, tag="work")
                for ch in range(2):
                    nc.tensor.matmul(
                        ps,
                        xT_sb[:, ch, jt * P:(jt + 1) * P],
                        gW_sb[:, ch, :],
                        start=(ch == 0), stop=(ch == 1))
                nc.vector.tensor_add(out=gx_sb[:, jt, :], in0=ps, in1=gbb_sb)

            # ---- stages B+C fused over j tiles ----
            # B: fT[j, i] = sum_d thetaT[d, j] * phiT[d, i]
            # C: yT[d', i] += sum_j gx[j, d'] * fT[j, i]
            for ih in range(2):
                psC = [psa.tile([P, 512], F32, tag="acc", name=f"psC{q}")
                       for q in range(4)]
                for jt in range(N // P):
                    fT = fTp.tile([P, 2, 512], F32R, tag="fT")
                    for ck in range(2):
                        ps = psw.tile([P, 512], F32, tag="work")
                        isl = slice(ih * 1024 + ck * 512,
                                    ih * 1024 + (ck + 1) * 512)
                        for dh in range(2):
                            nc.tensor.matmul(
                                ps,
                                thetaT_sb[:, dh, jt * P:(jt + 1) * P],
                                phiT_sb[:, dh, isl],
                                start=(dh == 0), stop=(dh == 1))
                        nc.vector.tensor_copy(out=fT[:, ck, :], in_=ps)
                    for dp in range(2):
                        for ck in range(2):
                            nc.tensor.matmul(
                                psC[dp * 2 + ck],
                                gx_sb[:, jt, dp * P:(dp + 1) * P],
                                fT[:, ck, :],
                                start=(jt == 0), stop=(jt == N // P - 1))
                for dp in range(2):
                    for ck in range(2):
                        isl = slice(ih * 1024 + ck * 512,
                                    ih * 1024 + (ck + 1) * 512)
                        nc.vector.tensor_copy(out=yT_sb[:, dp, isl],
                                              in_=psC[dp * 2 + ck])

            # ---- stage D: out[i, e] = yT.T @ WW + resid; DMA out ----
            out_r = out_d.ap().rearrange("(t p) d -> p t d", p=P)
            for it in range(HALF // P):
                ps = psw.tile([P, C], F32, tag="work")
                for dp in range(2):
                    nc.tensor.matmul(
                        ps,
                        yT_sb[:, dp, it * P:(it + 1) * P],
                        WW_sb[:, dp, :],
                        start=(dp == 0), stop=(dp == 1))
                nc.vector.tensor_add(out=resid_sb[:, it, :], in0=ps,
                                     in1=resid_sb[:, it, :])
                nc.sync.dma_start(out=out_r[:, it, :], in_=resid_sb[:, it, :])

    nc.finalize()
    return nc


def _get_module():
    if "nc" not in _CACHE:
        _CACHE["nc"] = _build_module()
    return _CACHE["nc"]


def _prep_in_maps(x, g_w, g_b, theta_w, theta_b, phi_w, phi_b, W_w, W_b):
    x = np.ascontiguousarray(np.asarray(x, dtype=np.float32))
    f32 = np.float32

    def col2(v):  # [256] -> [128, 2] (column h = channels h*128..h*128+127)
        return np.ascontiguousarray(np.asarray(v, f32).reshape(2, P).T)

    thW = np.ascontiguousarray(np.asarray(theta_w, f32).T)
    phW = np.ascontiguousarray(np.asarray(phi_w, f32).T)
    gW = np.ascontiguousarray(np.asarray(g_w, f32).T / N)
    WW = np.ascontiguousarray(np.asarray(W_w, f32).T)
    thb = col2(theta_b)
    phb = col2(phi_b)
    gbb = np.ascontiguousarray(
        np.broadcast_to(np.asarray(g_b, f32) / N, (P, C)))
    W_b = np.asarray(W_b, f32)

    in_maps = []
    for core in range(NCORES):
        b, h = core // 2, core % 2
        rows = slice(h * HALF, (h + 1) * HALF)
        xb = x[b]
        in_maps.append({
            "xT": np.ascontiguousarray(xb.T),
            "xTR": np.ascontiguousarray(xb[rows].T),
            "thW": thW, "phW": phW, "gW": gW, "WW": WW,
            "thb": thb, "phb": phb, "gbb": gbb,
            "resid": xb[rows] + W_b,
        })
    return in_maps


def kernel(x, g_w, g_b, theta_w, theta_b, phi_w, phi_b, W_w, W_b):
    nc = _get_module()
    in_maps = _prep_in_maps(x, g_w, g_b, theta_w, theta_b, phi_w, phi_b,
                            W_w, W_b)
    res = run_bass_kernel_spmd(nc, in_maps, core_ids=list(range(NCORES)))
    out = np.empty((B, N, C), dtype=np.float32)
    for core in range(NCORES):
        b, h = core // 2, core % 2
        out[b, h * HALF:(h + 1) * HALF, :] = res.results[core]["out"]
    return out
